# revision 25
# baseline (speedup 1.0000x reference)
"""Trainium2 Bass kernel: LayerNorm + multi-head self-attention + residual.

Computes, per batch b:
    xn = LayerNorm(x[b]) * g + b
    q/k/v = xn @ W{q,k,v}.T + b{q,k,v}      (16 heads, dh=64)
    attn  = softmax(q k^T + maskbias, over keys)
    out   = x + (attn @ (v*mask)) reshaped

Sharding over 8 cores: batch (2-way) x head-group (4-way, 4 heads each).
Each core gets full x[b] (for LayerNorm) plus its 256-column slice of the
Q/K/V weights, and produces a [2048, 256] slice of the output.

Host-side folding: LN's g is folded into the weight columns and LN's b into
the projection biases (Q = (x-mu)*rstd @ (W*g).T + (W@b + bq)), so the device
only computes the standardized activation xc = (x - mu) * rstd.

v2 restructure (from the v1 phase-serial kernel at 305.8us): the kernel is
engine-balance-bound: PE ~163us of matmul, ACT ~133us of EXP (the softmax
exp stream is irreducible: 16.8M elements through 128 lanes at 1.2GHz).
v1 serialized LN -> proj -> attention phases, leaving both engines ~55%
idle.  v2:
  1. ACT does (almost) nothing but EXP in the attention phase: the LN
     affine moved to GPSIMD (Pool; SBUF-only engine, otherwise idle), the
     attention epilogue copies to DVE.  xnT PSUM->SBUF copies stay on ACT
     but all land in phase I where ACT has slack.
  2. Phase I is chunk-pipelined: per 128-token chunk, LN stats (DVE) ->
     affine (Pool) -> PE transposes -> ACT copies -> V projection (PE);
     every 4 chunks the Q/K projections for that 512-column slice run.
     The PE transposes for a chunk pack into half a PSUM bank (f16), so
     one ACT copy moves 4 chunks' worth per instruction.
  3. Attention runs per (head, n-slice-of-1024): scores (PE, K=64), exp
     (ACT, [128,1024] per instruction), AV accumulate (PE) with psum
     pools sized so the score matmuls run ~2 tiles ahead of exp.
  4. Output epilogue: PE transpose, DVE reciprocal + fused (y*rec + x)
     scalar_tensor_tensor, one 3D-AP DMA per (head, n-slice).

Precision identical to v1 (matmuls fp16, softmax weights bf16, fp32
accumulation); validated ~5e-3 normalized absmax error vs fp32 reference.
"""

import sys

for _p in ("/opt/trn_rl_repo",):
    if _p not in sys.path:
        sys.path.insert(0, _p)

import numpy as np

import concourse.bacc as bacc
import concourse.bass as bass
import concourse.mybir as mybir
import concourse.tile as tile
from concourse.masks import make_identity

F32 = mybir.dt.float32
F16 = mybir.dt.float16
BF16 = mybir.dt.bfloat16

T = 2048          # sequence length
D = 1024          # model dim
HC = 4            # heads per core
DH = 64           # head dim
CC = HC * DH      # columns per core (256)
NC = T // 128     # 16 n/m chunks of 128
DC = D // 128     # 8 d chunks

_CACHE = {}


def _maybe_patch_ldw_opt():
    """Optionally re-enable walrus's redundant-LDWEIGHTS elimination.

    concourse hardcodes --enable-ldw-opt=false; our matmul streams reuse the
    stationary operand across consecutive matmuls, so the redundant weight
    loads are pure overhead. Gated by env until validated.
    """
    import os
    if os.environ.get("KERNEL_LDW_OPT") != "1" or _CACHE.get("ldw_patched"):
        return
    from concourse import bass_utils as _bu
    _orig = _bu.run_command

    def _run(argv, **kw):
        argv = ["--enable-ldw-opt=true" if a == "--enable-ldw-opt=false" else a
                for a in argv]
        return _orig(argv, **kw)

    _bu.run_command = _run
    _CACHE["ldw_patched"] = True


def build_bass():
    # Bacc (not plain Bass): its finalize() runs generate_event_semaphores,
    # which splits multi-waits into EventSemaphore instructions — walrus
    # rejects >1 sync wait on most engine instruction structs.
    nc = bacc.Bacc()

    x_d = nc.declare_dram_parameter("x", [T, D], F32, isOutput=False)
    xres_d = nc.declare_dram_parameter("xres", [T, CC], F32, isOutput=False)
    wqt_d = nc.declare_dram_parameter("wqt", [D, CC], F16, isOutput=False)
    wkt_d = nc.declare_dram_parameter("wkt", [D, CC], F16, isOutput=False)
    wvt_d = nc.declare_dram_parameter("wvt", [D, CC], F16, isOutput=False)
    bq_d = nc.declare_dram_parameter("bq2", [128, 2], F32, isOutput=False)
    bk_d = nc.declare_dram_parameter("bk2", [128, 2], F32, isOutput=False)
    bvr_d = nc.declare_dram_parameter("bvr", [1, CC], F16, isOutput=False)
    mb_d = nc.declare_dram_parameter("mbias", [128, NC], F32, isOutput=False)
    mm_d = nc.declare_dram_parameter("mmul", [128, NC], F32, isOutput=False)
    out_d = nc.declare_dram_parameter("out", [T, CC], F32, isOutput=True)

    with tile.TileContext(nc) as tc:
        _body(tc, x_d, xres_d, wqt_d, wkt_d, wvt_d,
              bq_d, bk_d, bvr_d, mb_d, mm_d, out_d)
    nc.finalize()
    return nc


def _body(tc, x_d, xres_d, wqt_d, wkt_d, wvt_d,
          bq_d, bk_d, bvr_d, mb_d, mm_d, out_d):
    nc = tc.nc
    import contextlib
    ctx = contextlib.ExitStack()
    with ctx:
        consts = ctx.enter_context(tc.tile_pool(name="consts", bufs=1))
        persist = ctx.enter_context(tc.tile_pool(name="persist", bufs=1))
        xcpool = ctx.enter_context(tc.tile_pool(name="xcpool", bufs=3))
        stats = ctx.enter_context(tc.tile_pool(name="stats", bufs=4))
        ppool = ctx.enter_context(tc.tile_pool(name="ppool", bufs=18))
        ytpool = ctx.enter_context(tc.tile_pool(name="ytpool", bufs=3))
        outpool = ctx.enter_context(tc.tile_pool(name="outpool", bufs=2))
        recpool = ctx.enter_context(tc.tile_pool(name="recpool", bufs=4))
        # PSUM budget (8 banks): sc 2x[128,1024] = 4, tmp 2x[128,512] = 2,
        # y 2x[128,512] = 2.
        scp = ctx.enter_context(tc.tile_pool(name="scp", bufs=2, space="PSUM"))
        tmpp = ctx.enter_context(tc.tile_pool(name="tmpp", bufs=2, space="PSUM"))
        yp = ctx.enter_context(tc.tile_pool(name="yp", bufs=2, space="PSUM"))

        # ---- DMAs: first x chunk, then the tiny constants (their "touch"
        # waits gate ACT's in-order stream), then weights, bulk x, and
        # finally xres (not read until the attention epilogues).
        x_all = persist.tile([128, NC, D], F32)
        xv = x_d[:].rearrange("(o p) d -> p o d", p=128)
        xsegs = [(0, 1), (1, 2), (2, 4), (4, 8), (8, 12), (12, 16)]
        for (a, b) in xsegs[:2]:
            nc.sync.dma_start(x_all[:, a:b, :], xv[:, a:b, :])

        bq_t = consts.tile([128, 2], F32)
        bk_t = consts.tile([128, 2], F32)
        nc.sync.dma_start(bq_t, bq_d[:])
        nc.sync.dma_start(bk_t, bk_d[:])
        bvr_t = consts.tile([1, CC], F16)
        nc.sync.dma_start(bvr_t, bvr_d[:])
        mb_t = consts.tile([128, NC], F32)
        mm_t = consts.tile([128, NC], F32)
        nc.sync.dma_start(mb_t, mb_d[:])
        nc.sync.dma_start(mm_t, mm_d[:])

        wq_sb = consts.tile([128, DC, CC], F16)
        wk_sb = consts.tile([128, DC, CC], F16)
        wv_sb = consts.tile([128, DC, CC], F16)
        nc.sync.dma_start(wv_sb, wvt_d[:].rearrange("(o p) c -> p o c", p=128))
        nc.sync.dma_start(wk_sb, wkt_d[:].rearrange("(o p) c -> p o c", p=128))
        nc.sync.dma_start(wq_sb, wqt_d[:].rearrange("(o p) c -> p o c", p=128))

        for (a, b) in xsegs[2:]:
            nc.sync.dma_start(x_all[:, a:b, :], xv[:, a:b, :])

        xres_all = persist.tile([128, NC, CC], F32)
        nc.sync.dma_start(xres_all,
                          xres_d[:].rearrange("(o p) c -> p o c", p=128))

        # absorb const-DMA completion waits on the engines that later read
        # these tiles via scalar-pointer operands (those instruction structs
        # can encode only one sync wait)
        touch_a = consts.tile([128, 1], F32)
        nc.scalar.copy(touch_a, bq_t[:, 0:1])
        nc.scalar.copy(touch_a, bk_t[:, 0:1])
        nc.scalar.copy(touch_a, mm_t[:, 0:1])
        nc.scalar.copy(touch_a, mb_t[:, 0:1])

        ident32 = consts.tile([128, 128], F32)
        make_identity(nc, ident32)
        ident16 = consts.tile([128, 128], F16)
        make_identity(nc, ident16)
        ones1 = consts.tile([1, 128], F16)
        nc.vector.memset(ones1, 1.0)

        # ---- persistent activations -----------------------------------
        xnT = persist.tile([128, DC, T], F16)       # xn^T (g,b folded on host)
        qT = persist.tile([128, 2, T], F16)         # Q^T per head-pair
        kT = persist.tile([128, 2, T], F16)
        vP = persist.tile([128, NC, HC * (DH + 1)], BF16)  # V' with ones cols

        # ones columns of V' (softmax denominator trick)
        vP4 = vP[:].rearrange("p i (h c) -> p i h c", c=DH + 1)
        nc.vector.memset(vP4[:, :, :, DH], 1.0)

        out_view = out_d[:].rearrange("(o p) c -> p o c", p=128)

        # ---- phase I: LN + transposes + projections, chunk-pipelined ----
        # LN rstd comes from a DVE-only fast-rsqrt (magic seed + 2 Newton
        # steps, batched per chunk-group) so ACT's in-order queue carries
        # only the xnT copies — an ACT sqrt would serialize the per-chunk
        # cross-engine chain at its full latency.
        sctp = None
        U32 = mybir.dt.uint32

        def ln_stats(mvg, j, ic):
            x_t = x_all[:, ic, :]
            st = stats.tile([128, 2, 6], F32, tag="st")
            nc.vector.bn_stats(st[:, 0, :], x_t[:, 0:512])
            nc.vector.bn_stats(st[:, 1, :], x_t[:, 512:1024])
            nc.vector.bn_aggr(mvg[:, j, :], st)

        def group_rstd(mvg, gw):
            # rstd = rsqrt(var+eps), nmr = mu*rstd for a group of gw chunks
            tv = stats.tile([128, 4], F32, tag="tv", name="tv")[:, 0:gw]
            nc.vector.tensor_scalar_add(tv, mvg[:, 0:gw, 1], 1e-5)
            ymag = stats.tile([128, 4], F32, tag="ymag", name="ymag")[:, 0:gw]
            yu = ymag.bitcast(U32)
            # seed bits = MAGIC - (i>>1); the subtract runs in the float
            # VALUE domain (u32->f32 convert, f32 affine, f32->u32 convert)
            # because DVE tensor_scalar imms/scalars are float-typed — the
            # <=32-ulp rounding this costs is far below the seed's own error.
            shf = stats.tile([128, 4], F32, tag="shf", name="shf")[:, 0:gw]
            nc.vector.tensor_scalar(
                out=shf.bitcast(U32), in0=tv.bitcast(U32), scalar1=1,
                scalar2=None, op0=mybir.AluOpType.logical_shift_right)
            nc.vector.tensor_copy(shf, shf.bitcast(U32))
            nc.vector.tensor_scalar(out=shf, in0=shf, scalar1=-1.0,
                                    scalar2=float(0x5F3759DF),
                                    op0=mybir.AluOpType.mult,
                                    op1=mybir.AluOpType.add)
            nc.vector.tensor_copy(yu, shf)
            t2 = stats.tile([128, 4], F32, tag="t2", name="t2")[:, 0:gw]
            for _ in range(2):
                nc.vector.tensor_tensor(out=t2, in0=ymag, in1=ymag,
                                        op=mybir.AluOpType.mult)
                nc.vector.tensor_tensor(out=t2, in0=t2, in1=tv,
                                        op=mybir.AluOpType.mult)
                nc.vector.tensor_scalar(out=t2, in0=t2, scalar1=-0.5,
                                        scalar2=1.5,
                                        op0=mybir.AluOpType.mult,
                                        op1=mybir.AluOpType.add)
                nc.vector.tensor_tensor(out=ymag, in0=ymag, in1=t2,
                                        op=mybir.AluOpType.mult)
            nmr = stats.tile([128, 4], F32, tag="nmr", name="nmr")[:, 0:gw]
            nc.vector.tensor_tensor(out=nmr, in0=mvg[:, 0:gw, 0], in1=ymag,
                                    op=mybir.AluOpType.mult)
            return ymag, nmr

        def ln_chunk(ic, rstd, murstd):
            nonlocal sctp
            x_t = x_all[:, ic, :]
            # standardize: xc = x*rstd - mu*rstd.  On DVE: its inputs are
            # DVE-internal (chain) + DMA, so this never blocks the pipeline;
            # GPSIMD (8 Q7 DSPs) measured ~15us per [128,1024] op — unusable.
            xc = xcpool.tile([128, D], F16, tag="xc")
            nc.vector.tensor_scalar(out=xc, in0=x_t, scalar1=rstd,
                                    scalar2=murstd,
                                    op0=mybir.AluOpType.mult,
                                    op1=mybir.AluOpType.subtract)
            # PE transposes: 8x [128,128] f16 packed into half of an sc tile
            if ic % 2 == 0:
                sctp = scp.tile([128, 1024], F32, tag="sc", name="sctp")
            scf = sctp.bitcast(F16)
            off = 1024 * (ic % 2)
            for dc in range(DC):
                nc.tensor.transpose(scf[:, off + 128 * dc:off + 128 * (dc + 1)],
                                    xc[:, 128 * dc:128 * (dc + 1)], ident16)
            # ACT copy PSUM->SBUF into xnT (all 8 d-chunks, one instruction)
            nc.scalar.copy(
                xnT[:, :, 128 * ic:128 * (ic + 1)],
                scf[:, off:off + 1024].rearrange("p (a b) -> p a b", b=128))
            # V projection for this chunk (+bias via rank-1 matmul, *mask).
            # The mask multiply runs on ACT (Copy with per-partition scale):
            # putting it on DVE would make DVE's in-order stream wait on the
            # PE->ACT chunk chain, stalling the next group's LN stats.
            psv = tmpp.tile([128, 512], F32, tag="tmp", name="psv")[:, 0:256]
            for dc in range(DC):
                nc.tensor.matmul(psv,
                                 lhsT=xnT[:, dc, 128 * ic:128 * (ic + 1)],
                                 rhs=wv_sb[:, dc, :],
                                 start=(dc == 0), stop=False)
            nc.tensor.matmul(psv, lhsT=ones1, rhs=bvr_t,
                             start=False, stop=True)
            nc.scalar.activation(
                out=vP4[:, ic, :, 0:DH],
                in_=psv.rearrange("p (h c) -> p h c", c=DH),
                func=mybir.ActivationFunctionType.Copy,
                scale=mm_t[:, ic:ic + 1])

        def qk_slice(s):
            # Q/K projections for n-columns [512s, 512(s+1)), both head-pairs.
            # Bias-add on ACT (Identity + per-partition bias) for the same
            # reason as the V mask multiply above.
            nsl = slice(512 * s, 512 * (s + 1))
            for w_sb, dstT, b_t in ((wk_sb, kT, bk_t), (wq_sb, qT, bq_t)):
                for pg in range(2):
                    pj = tmpp.tile([128, 512], F32, tag="tmp", name="pj")
                    for dc in range(DC):
                        nc.tensor.matmul(
                            pj, lhsT=w_sb[:, dc, 128 * pg:128 * (pg + 1)],
                            rhs=xnT[:, dc, nsl],
                            start=(dc == 0), stop=(dc == DC - 1))
                    nc.scalar.activation(
                        out=dstT[:, pg, nsl], in_=pj,
                        func=mybir.ActivationFunctionType.Identity,
                        bias=b_t[:, pg:pg + 1], scale=1.0)

        # ---- phase II: attention per (head, n-slice of 1024) ------------
        def score_exp(h, j2, ic):
            pg, hi = divmod(h, 2)
            rows = slice(64 * hi, 64 * hi + 64)
            n0 = 1024 * j2
            msl = slice(128 * ic, 128 * (ic + 1))
            scA = scp.tile([128, 1024], F32, tag="sc", name="scA")
            kA = kT[rows, pg, msl]
            nc.tensor.matmul(scA[:, 0:512], lhsT=kA,
                             rhs=qT[rows, pg, n0:n0 + 512],
                             start=True, stop=True)
            nc.tensor.matmul(scA[:, 512:1024], lhsT=kA,
                             rhs=qT[rows, pg, n0 + 512:n0 + 1024],
                             start=True, stop=True)
            pA = ppool.tile([128, 1024], BF16, tag="p")
            nc.scalar.activation(pA, scA,
                                 mybir.ActivationFunctionType.Exp,
                                 bias=mb_t[:, ic:ic + 1], scale=1.0)
            return pA

        def av_acc(h, ic, y0, y1, pA):
            vA = vP[:, ic, (DH + 1) * h:(DH + 1) * (h + 1)]
            nc.tensor.matmul(y0, lhsT=vA, rhs=pA[:, 0:512],
                             start=(ic == 0), stop=(ic == NC - 1))
            nc.tensor.matmul(y1, lhsT=vA, rhs=pA[:, 512:1024],
                             start=(ic == 0), stop=(ic == NC - 1))

        def attn_epilogue(h, j2, y0, y1):
            # y^T[65, 512] per half: transpose 128-col blocks, divide by the
            # denominator row, add residual, assemble one [128, 8, 64] tile
            out_t = outpool.tile([128, 8, DH], F32, tag="out")
            for half, y in ((0, y0), (1, y1)):
                yt = ytpool.tile([DH + 1, 512], F32, tag="yt")
                nc.vector.tensor_copy(yt, y)
                for k in range(4):
                    r = 4 * half + k
                    icg = 8 * j2 + r
                    otp = tmpp.tile([128, 512], F32, tag="tmp",
                                    name="otp")[:, 0:DH + 1]
                    nc.tensor.transpose(otp, yt[:, 128 * k:128 * (k + 1)],
                                        ident32[0:DH + 1, 0:DH + 1])
                    rec = recpool.tile([128, 1], F32, tag="rec")
                    nc.vector.reciprocal(rec, otp[:, DH:DH + 1])
                    nc.vector.scalar_tensor_tensor(
                        out=out_t[:, r, :], in0=otp[:, 0:DH], scalar=rec,
                        in1=xres_all[:, icg, DH * h:DH * (h + 1)],
                        op0=mybir.AluOpType.mult, op1=mybir.AluOpType.add)
            nc.sync.dma_start(
                out_view[:, 8 * j2:8 * (j2 + 1), DH * h:DH * (h + 1)], out_t)

        # Attention exps pulled into phase I: score+exp units (AV deferred
        # into phase II, where PE has slack under the ACT exp rail) for
        # groups whose kT/qT inputs exist.  Two pulled units are emitted
        # after each later chunk so ACT's in-order queue never delays the
        # xnT copies that gate the projections.
        pAs = {}
        pull_q = []
        for (a, b) in xsegs:
            gw = b - a
            mvg = stats.tile([128, 4, 2], F32, tag="mvg")
            for ic in range(a, b):
                ln_stats(mvg, ic - a, ic)
            rstdg, nmrg = group_rstd(mvg, gw)
            for ic in range(a, b):
                j = ic - a
                ln_chunk(ic, rstdg[:, j:j + 1], nmrg[:, j:j + 1])
                for u in [pull_q.pop(0) for _ in range(min(2, len(pull_q)))]:
                    pAs[u] = score_exp(*u)
                if ic % 4 == 3:
                    qk_slice(ic // 4)
            if b == 8:
                # qT[:, :, 0:1024] and kT chunks 0..7 now exist
                pull_q += [(0, 0, ic) for ic in range(0, 8)]
                pull_q += [(1, 0, ic) for ic in range(0, 8)]
        for u in pull_q:
            pAs[u] = score_exp(*u)

        # ---- phase II driver: one global pipeline -----------------------
        # Exps are emitted LAG units ahead of their AV matmuls (any AV
        # order within a group is fine: PSUM accumulation only needs the
        # start flag first and stop flag last, and PE executes in program
        # order).  Each group's epilogue is emitted right before the next
        # group's first AV, where the exp stream is already LAG ahead.
        LAG = 3
        units = [(h, j2, ic) for h in range(HC) for j2 in range(2)
                 for ic in range(NC)]
        ei = 0
        ys = None
        prev = None
        for k, u in enumerate(units):
            while ei < min(len(units), k + LAG):
                v = units[ei]
                if v not in pAs:
                    pAs[v] = score_exp(*v)
                ei += 1
            h, j2, ic = u
            if ic == 0:
                if prev is not None:
                    attn_epilogue(*prev)
                y0 = yp.tile([128, 512], F32, tag="y", name="y0")[0:DH + 1]
                y1 = yp.tile([128, 512], F32, tag="y", name="y1")[0:DH + 1]
                ys = (y0, y1)
                prev = (h, j2, y0, y1)
            av_acc(h, ic, ys[0], ys[1], pAs.pop(u))
        attn_epilogue(*prev)


def _host_in_map(core, x, src_mask, ln_g, ln_b, Wq, bq, Wk, bk, Wv, bv):
    b, hg = divmod(core, 4)
    cs = CC * hg
    xb = np.ascontiguousarray(x[b], dtype=np.float32)
    mask = np.asarray(src_mask[b, :, 0], dtype=np.float32)
    ln_g = np.asarray(ln_g, np.float32)
    ln_b = np.asarray(ln_b, np.float32)

    def wfold(W):
        # fold LN scale g into weight columns: (W * g).T, fp16
        Ws = np.asarray(W, np.float32)[cs:cs + CC, :]
        return np.ascontiguousarray((Ws * ln_g[None, :]).T).astype(np.float16)

    def bfold(W, bb):
        # fold LN shift b into the projection bias: W @ b + bias
        Ws = np.asarray(W, np.float32)[cs:cs + CC, :]
        return Ws @ ln_b + np.asarray(bb, np.float32)[cs:cs + CC]

    return {
        "x": xb,
        "xres": np.ascontiguousarray(xb[:, cs:cs + CC]),
        "wqt": wfold(Wq),
        "wkt": wfold(Wk),
        "wvt": wfold(Wv),
        "bq2": np.ascontiguousarray(bfold(Wq, bq).reshape(2, 128).T),
        "bk2": np.ascontiguousarray(bfold(Wk, bk).reshape(2, 128).T),
        "bvr": bfold(Wv, bv).reshape(1, CC).astype(np.float16),
        "mbias": np.ascontiguousarray(
            ((1.0 - mask) * -1000000.0).reshape(NC, 128).T),
        "mmul": np.ascontiguousarray(mask.reshape(NC, 128).T),
    }


def kernel(x, src_mask, ln_g, ln_b, Wq, bq, Wk, bk, Wv, bv, _trace=False,
           _tmpdir=None):
    x = np.asarray(x, dtype=np.float32)
    B = x.shape[0]
    _maybe_patch_ldw_opt()
    if "nc" not in _CACHE:
        _CACHE["nc"] = build_bass()
    nc = _CACHE["nc"]

    from concourse.bass_utils import run_bass_kernel_spmd
    in_maps = [
        _host_in_map(c, x, np.asarray(src_mask), np.asarray(ln_g),
                     np.asarray(ln_b), np.asarray(Wq), np.asarray(bq),
                     np.asarray(Wk), np.asarray(bk), np.asarray(Wv),
                     np.asarray(bv))
        for c in range(8)
    ]
    res = run_bass_kernel_spmd(nc, in_maps, core_ids=list(range(8)),
                               trace=_trace, tmpdir=_tmpdir)
    out = np.empty((B, T, D), dtype=np.float32)
    for c in range(8):
        b, hg = divmod(c, 4)
        out[b, :, CC * hg:CC * (hg + 1)] = res.results[c]["out"]
    if _trace:
        _CACHE["last_result"] = res
    return out


# revision 26
# speedup vs baseline: 1.2038x; 1.2038x over previous
"""Trainium2 Bass kernel: LayerNorm + multi-head self-attention + residual.

Computes, per batch b:
    xn = LayerNorm(x[b]) * g + b
    q/k/v = xn @ W{q,k,v}.T + b{q,k,v}      (16 heads, dh=64)
    attn  = softmax(q k^T + maskbias, over keys)
    out   = x + (attn @ (v*mask)) reshaped

Sharding over 8 cores: batch (2-way) x head-group (4-way, 4 heads each).
Each core gets full x[b] (for LayerNorm) plus its 256-column slice of the
Q/K/V weights, and produces a [2048, 256] slice of the output.

Host-side folding: LN's g is folded into the weight columns and LN's b into
the projection biases (Q = (x-mu)*rstd @ (W*g).T + (W@b + bq)), so the device
only computes the standardized activation xc = (x - mu) * rstd.

v2 restructure (from the v1 phase-serial kernel at 305.8us): the kernel is
engine-balance-bound: PE ~163us of matmul, ACT ~133us of EXP (the softmax
exp stream is irreducible: 16.8M elements through 128 lanes at 1.2GHz).
v1 serialized LN -> proj -> attention phases, leaving both engines ~55%
idle.  v2:
  1. ACT does (almost) nothing but EXP in the attention phase: the LN
     affine moved to GPSIMD (Pool; SBUF-only engine, otherwise idle), the
     attention epilogue copies to DVE.  xnT PSUM->SBUF copies stay on ACT
     but all land in phase I where ACT has slack.
  2. Phase I is chunk-pipelined: per 128-token chunk, LN stats (DVE) ->
     affine (Pool) -> PE transposes -> ACT copies -> V projection (PE);
     every 4 chunks the Q/K projections for that 512-column slice run.
     The PE transposes for a chunk pack into half a PSUM bank (f16), so
     one ACT copy moves 4 chunks' worth per instruction.
  3. Attention runs per (head, n-slice-of-1024): scores (PE, K=64), exp
     (ACT, [128,1024] per instruction), AV accumulate (PE) with psum
     pools sized so the score matmuls run ~2 tiles ahead of exp.
  4. Output epilogue: PE transpose, DVE reciprocal + fused (y*rec + x)
     scalar_tensor_tensor, one 3D-AP DMA per (head, n-slice).

Precision identical to v1 (matmuls fp16, softmax weights bf16, fp32
accumulation); validated ~5e-3 normalized absmax error vs fp32 reference.
"""

import sys

for _p in ("/opt/trn_rl_repo",):
    if _p not in sys.path:
        sys.path.insert(0, _p)

import numpy as np

import concourse.bacc as bacc
import concourse.bass as bass
import concourse.mybir as mybir
import concourse.tile as tile
from concourse.masks import make_identity

F32 = mybir.dt.float32
F16 = mybir.dt.float16
BF16 = mybir.dt.bfloat16

T = 2048          # sequence length
D = 1024          # model dim
HC = 4            # heads per core
DH = 64           # head dim
CC = HC * DH      # columns per core (256)
NC = T // 128     # 16 n/m chunks of 128
DC = D // 128     # 8 d chunks

_CACHE = {}


def _maybe_patch_ldw_opt():
    """Optionally re-enable walrus's redundant-LDWEIGHTS elimination.

    concourse hardcodes --enable-ldw-opt=false; our matmul streams reuse the
    stationary operand across consecutive matmuls, so the redundant weight
    loads are pure overhead. Gated by env until validated.
    """
    import os
    if os.environ.get("KERNEL_LDW_OPT") != "1" or _CACHE.get("ldw_patched"):
        return
    from concourse import bass_utils as _bu
    _orig = _bu.run_command

    def _run(argv, **kw):
        argv = ["--enable-ldw-opt=true" if a == "--enable-ldw-opt=false" else a
                for a in argv]
        return _orig(argv, **kw)

    _bu.run_command = _run
    _CACHE["ldw_patched"] = True


def build_bass():
    # Bacc (not plain Bass): its finalize() runs generate_event_semaphores,
    # which splits multi-waits into EventSemaphore instructions — walrus
    # rejects >1 sync wait on most engine instruction structs.
    nc = bacc.Bacc()

    x_d = nc.declare_dram_parameter("x", [T, D], F32, isOutput=False)
    xres_d = nc.declare_dram_parameter("xres", [T, CC], F32, isOutput=False)
    wqt_d = nc.declare_dram_parameter("wqt", [D, CC], F16, isOutput=False)
    wkt_d = nc.declare_dram_parameter("wkt", [D, CC], F16, isOutput=False)
    wvt_d = nc.declare_dram_parameter("wvt", [D, CC], F16, isOutput=False)
    bq_d = nc.declare_dram_parameter("bq2", [128, 2], F32, isOutput=False)
    bk_d = nc.declare_dram_parameter("bk2", [128, 2], F32, isOutput=False)
    bvr_d = nc.declare_dram_parameter("bvr", [1, CC], F16, isOutput=False)
    mb_d = nc.declare_dram_parameter("mbias", [128, NC], F32, isOutput=False)
    mm_d = nc.declare_dram_parameter("mmul", [128, NC], F32, isOutput=False)
    out_d = nc.declare_dram_parameter("out", [T, CC], F32, isOutput=True)

    with tile.TileContext(nc) as tc:
        _body(tc, x_d, xres_d, wqt_d, wkt_d, wvt_d,
              bq_d, bk_d, bvr_d, mb_d, mm_d, out_d)
    nc.finalize()
    return nc


def _body(tc, x_d, xres_d, wqt_d, wkt_d, wvt_d,
          bq_d, bk_d, bvr_d, mb_d, mm_d, out_d):
    nc = tc.nc
    import contextlib
    ctx = contextlib.ExitStack()
    with ctx:
        consts = ctx.enter_context(tc.tile_pool(name="consts", bufs=1))
        persist = ctx.enter_context(tc.tile_pool(name="persist", bufs=1))
        xcpool = ctx.enter_context(tc.tile_pool(name="xcpool", bufs=3))
        stats = ctx.enter_context(tc.tile_pool(name="stats", bufs=4))
        ppool = ctx.enter_context(tc.tile_pool(name="ppool", bufs=16))
        ytpool = ctx.enter_context(tc.tile_pool(name="ytpool", bufs=3))
        outpool = ctx.enter_context(tc.tile_pool(name="outpool", bufs=2))
        recpool = ctx.enter_context(tc.tile_pool(name="recpool", bufs=4))
        # PSUM budget (8 banks): sc 2x[128,1024] = 4, tmp 2x[128,512] = 2,
        # y 2x[128,512] = 2.
        scp = ctx.enter_context(tc.tile_pool(name="scp", bufs=2, space="PSUM"))
        tmpp = ctx.enter_context(tc.tile_pool(name="tmpp", bufs=2, space="PSUM"))
        yp = ctx.enter_context(tc.tile_pool(name="yp", bufs=2, space="PSUM"))

        # ---- DMAs: first x chunk, then the tiny constants (their "touch"
        # waits gate ACT's in-order stream), then weights, bulk x, and
        # finally xres (not read until the attention epilogues).
        x_all = persist.tile([128, NC, D], F32)
        xv = x_d[:].rearrange("(o p) d -> p o d", p=128)
        xsegs = [(0, 1), (1, 2), (2, 4), (4, 8), (8, 12), (12, 16)]
        for (a, b) in xsegs[:2]:
            nc.sync.dma_start(x_all[:, a:b, :], xv[:, a:b, :])

        bq_t = consts.tile([128, 2], F32)
        bk_t = consts.tile([128, 2], F32)
        nc.sync.dma_start(bq_t, bq_d[:])
        nc.sync.dma_start(bk_t, bk_d[:])
        bvr_t = consts.tile([1, CC], F16)
        nc.sync.dma_start(bvr_t, bvr_d[:])
        mb_t = consts.tile([128, NC], F32)
        mm_t = consts.tile([128, NC], F32)
        nc.sync.dma_start(mb_t, mb_d[:])
        nc.sync.dma_start(mm_t, mm_d[:])

        wq_sb = consts.tile([128, DC, CC], F16)
        wk_sb = consts.tile([128, DC, CC], F16)
        wv_sb = consts.tile([128, DC, CC], F16)
        nc.sync.dma_start(wv_sb, wvt_d[:].rearrange("(o p) c -> p o c", p=128))
        nc.sync.dma_start(wk_sb, wkt_d[:].rearrange("(o p) c -> p o c", p=128))
        nc.sync.dma_start(wq_sb, wqt_d[:].rearrange("(o p) c -> p o c", p=128))

        for (a, b) in xsegs[2:]:
            nc.sync.dma_start(x_all[:, a:b, :], xv[:, a:b, :])

        xres_all = persist.tile([128, NC, CC], F32)
        nc.sync.dma_start(xres_all,
                          xres_d[:].rearrange("(o p) c -> p o c", p=128))

        # absorb const-DMA completion waits on the engines that later read
        # these tiles via scalar-pointer operands (those instruction structs
        # can encode only one sync wait)
        touch_a = consts.tile([128, 1], F32)
        nc.scalar.copy(touch_a, bq_t[:, 0:1])
        nc.scalar.copy(touch_a, bk_t[:, 0:1])
        nc.scalar.copy(touch_a, mm_t[:, 0:1])
        nc.scalar.copy(touch_a, mb_t[:, 0:1])

        ident32 = consts.tile([128, 128], F32)
        make_identity(nc, ident32)
        ident16 = consts.tile([128, 128], F16)
        make_identity(nc, ident16)
        ones1 = consts.tile([1, 128], F16)
        nc.vector.memset(ones1, 1.0)

        # ---- persistent activations -----------------------------------
        xnT = persist.tile([128, DC, T], F16)       # xn^T (g,b folded on host)
        qT = persist.tile([128, 2, T], F16)         # Q^T per head-pair
        kT = persist.tile([128, 2, T], F16)
        vP = persist.tile([128, NC, HC * (DH + 1)], BF16)  # V' with ones cols

        # ones columns of V' (softmax denominator trick)
        vP4 = vP[:].rearrange("p i (h c) -> p i h c", c=DH + 1)
        nc.vector.memset(vP4[:, :, :, DH], 1.0)

        out_view = out_d[:].rearrange("(o p) c -> p o c", p=128)

        # ---- phase I: LN + transposes + projections, chunk-pipelined ----
        # LN rstd comes from a DVE-only fast-rsqrt (magic seed + 2 Newton
        # steps, batched per chunk-group) so ACT's in-order queue carries
        # only the xnT copies — an ACT sqrt would serialize the per-chunk
        # cross-engine chain at its full latency.
        sctp = None
        U32 = mybir.dt.uint32

        def ln_stats(mvg, j, ic):
            x_t = x_all[:, ic, :]
            st = stats.tile([128, 2, 6], F32, tag="st")
            nc.vector.bn_stats(st[:, 0, :], x_t[:, 0:512])
            nc.vector.bn_stats(st[:, 1, :], x_t[:, 512:1024])
            nc.vector.bn_aggr(mvg[:, j, :], st)

        def group_rstd(mvg, gw):
            # rstd = rsqrt(var+eps), nmr = mu*rstd for a group of gw chunks
            tv = stats.tile([128, 4], F32, tag="tv", name="tv")[:, 0:gw]
            nc.vector.tensor_scalar_add(tv, mvg[:, 0:gw, 1], 1e-5)
            ymag = stats.tile([128, 4], F32, tag="ymag", name="ymag")[:, 0:gw]
            yu = ymag.bitcast(U32)
            # seed bits = MAGIC - (i>>1); the subtract runs in the float
            # VALUE domain (u32->f32 convert, f32 affine, f32->u32 convert)
            # because DVE tensor_scalar imms/scalars are float-typed — the
            # <=32-ulp rounding this costs is far below the seed's own error.
            shf = stats.tile([128, 4], F32, tag="shf", name="shf")[:, 0:gw]
            nc.vector.tensor_scalar(
                out=shf.bitcast(U32), in0=tv.bitcast(U32), scalar1=1,
                scalar2=None, op0=mybir.AluOpType.logical_shift_right)
            nc.vector.tensor_copy(shf, shf.bitcast(U32))
            nc.vector.tensor_scalar(out=shf, in0=shf, scalar1=-1.0,
                                    scalar2=float(0x5F3759DF),
                                    op0=mybir.AluOpType.mult,
                                    op1=mybir.AluOpType.add)
            nc.vector.tensor_copy(yu, shf)
            t2 = stats.tile([128, 4], F32, tag="t2", name="t2")[:, 0:gw]
            for _ in range(2):
                nc.vector.tensor_tensor(out=t2, in0=ymag, in1=ymag,
                                        op=mybir.AluOpType.mult)
                nc.vector.tensor_tensor(out=t2, in0=t2, in1=tv,
                                        op=mybir.AluOpType.mult)
                nc.vector.tensor_scalar(out=t2, in0=t2, scalar1=-0.5,
                                        scalar2=1.5,
                                        op0=mybir.AluOpType.mult,
                                        op1=mybir.AluOpType.add)
                nc.vector.tensor_tensor(out=ymag, in0=ymag, in1=t2,
                                        op=mybir.AluOpType.mult)
            nmr = stats.tile([128, 4], F32, tag="nmr", name="nmr")[:, 0:gw]
            nc.vector.tensor_tensor(out=nmr, in0=mvg[:, 0:gw, 0], in1=ymag,
                                    op=mybir.AluOpType.mult)
            return ymag, nmr

        def ln_chunk(ic, rstd, murstd):
            nonlocal sctp
            x_t = x_all[:, ic, :]
            # standardize: xc = x*rstd - mu*rstd.  On DVE: its inputs are
            # DVE-internal (chain) + DMA, so this never blocks the pipeline;
            # GPSIMD (8 Q7 DSPs) measured ~15us per [128,1024] op — unusable.
            xc = xcpool.tile([128, D], F16, tag="xc")
            nc.vector.tensor_scalar(out=xc, in0=x_t, scalar1=rstd,
                                    scalar2=murstd,
                                    op0=mybir.AluOpType.mult,
                                    op1=mybir.AluOpType.subtract)
            # PE transposes: 8x [128,128] f16 packed into half of an sc tile
            if ic % 2 == 0:
                sctp = scp.tile([128, 1024], F32, tag="sc", name="sctp")
            scf = sctp.bitcast(F16)
            off = 1024 * (ic % 2)
            for dc in range(DC):
                nc.tensor.transpose(scf[:, off + 128 * dc:off + 128 * (dc + 1)],
                                    xc[:, 128 * dc:128 * (dc + 1)], ident16)
            # ACT copy PSUM->SBUF into xnT (all 8 d-chunks, one instruction)
            nc.scalar.copy(
                xnT[:, :, 128 * ic:128 * (ic + 1)],
                scf[:, off:off + 1024].rearrange("p (a b) -> p a b", b=128))
            # V projection for this chunk (+bias via rank-1 matmul, *mask).
            # The mask multiply runs on ACT (Copy with per-partition scale):
            # putting it on DVE would make DVE's in-order stream wait on the
            # PE->ACT chunk chain, stalling the next group's LN stats.
            psv = tmpp.tile([128, 512], F32, tag="tmp", name="psv")[:, 0:256]
            for dc in range(DC):
                nc.tensor.matmul(psv,
                                 lhsT=xnT[:, dc, 128 * ic:128 * (ic + 1)],
                                 rhs=wv_sb[:, dc, :],
                                 start=(dc == 0), stop=False)
            nc.tensor.matmul(psv, lhsT=ones1, rhs=bvr_t,
                             start=False, stop=True)
            nc.scalar.activation(
                out=vP4[:, ic, :, 0:DH],
                in_=psv.rearrange("p (h c) -> p h c", c=DH),
                func=mybir.ActivationFunctionType.Copy,
                scale=mm_t[:, ic:ic + 1])

        def qk_slice(s):
            # Q/K projections for n-columns [512s, 512(s+1)), both head-pairs.
            # Bias-add on ACT (Identity + per-partition bias) for the same
            # reason as the V mask multiply above.
            nsl = slice(512 * s, 512 * (s + 1))
            for w_sb, dstT, b_t in ((wk_sb, kT, bk_t), (wq_sb, qT, bq_t)):
                for pg in range(2):
                    pj = tmpp.tile([128, 512], F32, tag="tmp", name="pj")
                    for dc in range(DC):
                        nc.tensor.matmul(
                            pj, lhsT=w_sb[:, dc, 128 * pg:128 * (pg + 1)],
                            rhs=xnT[:, dc, nsl],
                            start=(dc == 0), stop=(dc == DC - 1))
                    nc.scalar.activation(
                        out=dstT[:, pg, nsl], in_=pj,
                        func=mybir.ActivationFunctionType.Identity,
                        bias=b_t[:, pg:pg + 1], scale=1.0)

        # ---- phase II: attention per (head, n-slice of 1024) ------------
        def score_exp(h, j2, ic):
            pg, hi = divmod(h, 2)
            rows = slice(64 * hi, 64 * hi + 64)
            n0 = 1024 * j2
            msl = slice(128 * ic, 128 * (ic + 1))
            scA = scp.tile([128, 1024], F32, tag="sc", name="scA")
            kA = kT[rows, pg, msl]
            nc.tensor.matmul(scA[:, 0:512], lhsT=kA,
                             rhs=qT[rows, pg, n0:n0 + 512],
                             start=True, stop=True)
            nc.tensor.matmul(scA[:, 512:1024], lhsT=kA,
                             rhs=qT[rows, pg, n0 + 512:n0 + 1024],
                             start=True, stop=True)
            pA = ppool.tile([128, 1024], BF16, tag="p")
            nc.scalar.activation(pA, scA,
                                 mybir.ActivationFunctionType.Exp,
                                 bias=mb_t[:, ic:ic + 1], scale=1.0)
            return pA

        def av_acc(h, ic, y0, y1, pA):
            vA = vP[:, ic, (DH + 1) * h:(DH + 1) * (h + 1)]
            nc.tensor.matmul(y0, lhsT=vA, rhs=pA[:, 0:512],
                             start=(ic == 0), stop=(ic == NC - 1))
            nc.tensor.matmul(y1, lhsT=vA, rhs=pA[:, 512:1024],
                             start=(ic == 0), stop=(ic == NC - 1))

        def attn_epilogue(h, j2, y0, y1):
            # y^T[65, 512] per half: transpose 128-col blocks, divide by the
            # denominator row, add residual, assemble one [128, 8, 64] tile
            out_t = outpool.tile([128, 8, DH], F32, tag="out")
            for half, y in ((0, y0), (1, y1)):
                yt = ytpool.tile([DH + 1, 512], F32, tag="yt")
                nc.vector.tensor_copy(yt, y)
                for k in range(4):
                    r = 4 * half + k
                    icg = 8 * j2 + r
                    otp = tmpp.tile([128, 512], F32, tag="tmp",
                                    name="otp")[:, 0:DH + 1]
                    nc.tensor.transpose(otp, yt[:, 128 * k:128 * (k + 1)],
                                        ident32[0:DH + 1, 0:DH + 1])
                    rec = recpool.tile([128, 1], F32, tag="rec")
                    nc.vector.reciprocal(rec, otp[:, DH:DH + 1])
                    nc.vector.scalar_tensor_tensor(
                        out=out_t[:, r, :], in0=otp[:, 0:DH], scalar=rec,
                        in1=xres_all[:, icg, DH * h:DH * (h + 1)],
                        op0=mybir.AluOpType.mult, op1=mybir.AluOpType.add)
            nc.sync.dma_start(
                out_view[:, 8 * j2:8 * (j2 + 1), DH * h:DH * (h + 1)], out_t)

        # Attention exps pulled into phase I: score+exp units (AV deferred
        # into phase II, where PE has slack under the ACT exp rail) for
        # groups whose kT/qT inputs exist.  Two pulled units are emitted
        # after each later chunk so ACT's in-order queue never delays the
        # xnT copies that gate the projections.
        pAs = {}
        pull_q = []
        for (a, b) in xsegs:
            gw = b - a
            mvg = stats.tile([128, 4, 2], F32, tag="mvg")
            for ic in range(a, b):
                ln_stats(mvg, ic - a, ic)
            rstdg, nmrg = group_rstd(mvg, gw)
            for ic in range(a, b):
                j = ic - a
                ln_chunk(ic, rstdg[:, j:j + 1], nmrg[:, j:j + 1])
                for u in [pull_q.pop(0) for _ in range(min(2, len(pull_q)))]:
                    pAs[u] = score_exp(*u)
                if ic % 4 == 3:
                    qk_slice(ic // 4)
            if b == 8:
                # qT[:, :, 0:1024] and kT chunks 0..7 now exist
                pull_q += [(0, 0, ic) for ic in range(0, 6)]
                pull_q += [(1, 0, ic) for ic in range(0, 6)]
        for u in pull_q:
            pAs[u] = score_exp(*u)

        # ---- phase II driver: one global pipeline -----------------------
        # Exps are emitted LAG units ahead of their AV matmuls (any AV
        # order within a group is fine: PSUM accumulation only needs the
        # start flag first and stop flag last, and PE executes in program
        # order).  Each group's epilogue is emitted right before the next
        # group's first AV, where the exp stream is already LAG ahead.
        LAG = 3
        units = [(h, j2, ic) for h in range(HC) for j2 in range(2)
                 for ic in range(NC)]
        ei = 0
        ys = None
        prev = None
        for k, u in enumerate(units):
            while ei < min(len(units), k + LAG):
                v = units[ei]
                if v not in pAs:
                    pAs[v] = score_exp(*v)
                ei += 1
            h, j2, ic = u
            if ic == 0:
                if prev is not None:
                    attn_epilogue(*prev)
                y0 = yp.tile([128, 512], F32, tag="y", name="y0")[0:DH + 1]
                y1 = yp.tile([128, 512], F32, tag="y", name="y1")[0:DH + 1]
                ys = (y0, y1)
                prev = (h, j2, y0, y1)
            av_acc(h, ic, ys[0], ys[1], pAs.pop(u))
        attn_epilogue(*prev)


def _host_in_map(core, x, src_mask, ln_g, ln_b, Wq, bq, Wk, bk, Wv, bv):
    b, hg = divmod(core, 4)
    cs = CC * hg
    xb = np.ascontiguousarray(x[b], dtype=np.float32)
    mask = np.asarray(src_mask[b, :, 0], dtype=np.float32)
    ln_g = np.asarray(ln_g, np.float32)
    ln_b = np.asarray(ln_b, np.float32)

    def wfold(W):
        # fold LN scale g into weight columns: (W * g).T, fp16
        Ws = np.asarray(W, np.float32)[cs:cs + CC, :]
        return np.ascontiguousarray((Ws * ln_g[None, :]).T).astype(np.float16)

    def bfold(W, bb):
        # fold LN shift b into the projection bias: W @ b + bias
        Ws = np.asarray(W, np.float32)[cs:cs + CC, :]
        return Ws @ ln_b + np.asarray(bb, np.float32)[cs:cs + CC]

    return {
        "x": xb,
        "xres": np.ascontiguousarray(xb[:, cs:cs + CC]),
        "wqt": wfold(Wq),
        "wkt": wfold(Wk),
        "wvt": wfold(Wv),
        "bq2": np.ascontiguousarray(bfold(Wq, bq).reshape(2, 128).T),
        "bk2": np.ascontiguousarray(bfold(Wk, bk).reshape(2, 128).T),
        "bvr": bfold(Wv, bv).reshape(1, CC).astype(np.float16),
        "mbias": np.ascontiguousarray(
            ((1.0 - mask) * -1000000.0).reshape(NC, 128).T),
        "mmul": np.ascontiguousarray(mask.reshape(NC, 128).T),
    }


def kernel(x, src_mask, ln_g, ln_b, Wq, bq, Wk, bk, Wv, bv, _trace=False,
           _tmpdir=None):
    x = np.asarray(x, dtype=np.float32)
    B = x.shape[0]
    _maybe_patch_ldw_opt()
    if "nc" not in _CACHE:
        _CACHE["nc"] = build_bass()
    nc = _CACHE["nc"]

    from concourse.bass_utils import run_bass_kernel_spmd
    in_maps = [
        _host_in_map(c, x, np.asarray(src_mask), np.asarray(ln_g),
                     np.asarray(ln_b), np.asarray(Wq), np.asarray(bq),
                     np.asarray(Wk), np.asarray(bk), np.asarray(Wv),
                     np.asarray(bv))
        for c in range(8)
    ]
    res = run_bass_kernel_spmd(nc, in_maps, core_ids=list(range(8)),
                               trace=_trace, tmpdir=_tmpdir)
    out = np.empty((B, T, D), dtype=np.float32)
    for c in range(8):
        b, hg = divmod(c, 4)
        out[b, :, CC * hg:CC * (hg + 1)] = res.results[c]["out"]
    if _trace:
        _CACHE["last_result"] = res
    return out
